# revision 1
# baseline (speedup 1.0000x reference)
"""Trainium2 Bass kernel for nn_InvariantHeadviaTP.

Reference computation (after dead-code elimination -- y1/y2/gates are never
used by the output):
    x0   = node_vec[:, :128]                  # [N, 128]
    a    = node_embedding                     # [N, 16]
    s0   = einsum('ni,na,iak->nk', x0, a, W1_l0[:, :, :128]) / sqrt(2048) + b1[:128]
    scal = silu(s0)                           # [N, 128]
    mid  = einsum('ni,na,iak->nk', scal, a, W2) / sqrt(2048) + b2   # [N, 16]
    h    = silu(mid @ W3 / 4 + b3)            # [N, 16]
    out  = h @ W4 / 4 + b4                    # [N, 1]

Strategy: data-parallel over 8 cores (2048 nodes each). Per core, work in a
transposed layout: features on SBUF partitions, nodes on the free dim.
The bilinear contractions over c=(a,i) [16*128=2048] are done as 16
PSUM-accumulated matmuls with bf16 operands:
    s0T[k, n] = sum_a sum_i W[i,a,k] * (x0T[i,n] * aT[a,n])
The inner operand U_a[i,n] = x0T[i,n]*aT[a,n] is built on the vector engine
as a tensor_tensor multiply against "Arep_a" = row a of aT broadcast across
128 partitions; Arep is produced on the tensor engine via a one-hot selector
matmul (out = sel_a.T @ aT = broadcast).
"""

import numpy as np
import ml_dtypes
from contextlib import ExitStack

import concourse.bass as bass
import concourse.mybir as mybir
import concourse.tile as tile
from concourse import bacc
from concourse.bass import ts
from concourse.bass_utils import run_bass_kernel_spmd

N_CORES = 8
N_FULL = 16384
NSH = N_FULL // N_CORES          # 2048 nodes per core
A = 16                           # attr dim
M0 = 128                         # MUL0 (scalar channels)
FREE = 512                       # node tile (free dim) per inner step
SCALE = 1.0 / np.sqrt(M0 * A)    # path normalization of both fctp einsums
BF16 = ml_dtypes.bfloat16

AF = mybir.ActivationFunctionType
F32 = mybir.dt.float32
DBF16 = mybir.dt.bfloat16


def build_nc(nsh: int = NSH, free: int = FREE, num_devices: int = N_CORES):
    nc = bacc.Bacc(
        "TRN2",
        target_bir_lowering=False,
        debug=False,
        enable_asserts=False,
        num_devices=num_devices,
    )

    x0t = nc.dram_tensor("x0t", [M0, nsh], DBF16, kind="ExternalInput").ap()
    atrep = nc.dram_tensor("atrep", [A * M0, nsh], DBF16, kind="ExternalInput").ap()
    w0 = nc.dram_tensor("w0", [M0, A * M0], DBF16, kind="ExternalInput").ap()
    w2 = nc.dram_tensor("w2", [M0, A * A], DBF16, kind="ExternalInput").ap()
    w3 = nc.dram_tensor("w3", [A, A], DBF16, kind="ExternalInput").ap()
    w4 = nc.dram_tensor("w4", [A, 1], DBF16, kind="ExternalInput").ap()
    b1 = nc.dram_tensor("b1", [M0, 1], F32, kind="ExternalInput").ap()
    b2 = nc.dram_tensor("b2", [A, 1], F32, kind="ExternalInput").ap()
    b3 = nc.dram_tensor("b3", [A, 1], F32, kind="ExternalInput").ap()
    b4 = nc.dram_tensor("b4", [1, 1], F32, kind="ExternalInput").ap()
    outt = nc.dram_tensor("outt", [1, nsh], F32, kind="ExternalOutput").ap()

    n_tiles = nsh // free

    with tile.TileContext(nc) as tc, ExitStack() as ctx:
        consts = ctx.enter_context(tc.tile_pool(name="consts", bufs=1))

        x0t_sb = consts.tile([M0, nsh], DBF16)
        nc.sync.dma_start(x0t_sb[:], x0t)
        w0_sb = consts.tile([M0, A * M0], DBF16)
        nc.sync.dma_start(w0_sb[:], w0)
        w2_sb = consts.tile([M0, A * A], DBF16)
        nc.sync.dma_start(w2_sb[:], w2)
        w3_sb = consts.tile([A, A], DBF16)
        nc.sync.dma_start(w3_sb[:], w3)
        w4_sb = consts.tile([A, 1], DBF16)
        nc.sync.dma_start(w4_sb[:], w4)
        b1_sb = consts.tile([M0, 1], F32)
        nc.sync.dma_start(b1_sb[:], b1)
        b2_sb = consts.tile([A, 1], F32)
        nc.sync.dma_start(b2_sb[:], b2)
        b3_sb = consts.tile([A, 1], F32)
        nc.sync.dma_start(b3_sb[:], b3)
        b4_sb = consts.tile([1, 1], F32)
        nc.sync.dma_start(b4_sb[:], b4)

        arep_pool = ctx.enter_context(tc.tile_pool(name="arep", bufs=2))
        u_pool = ctx.enter_context(tc.tile_pool(name="u", bufs=3))
        s_pool = ctx.enter_context(tc.tile_pool(name="s", bufs=2))
        o_pool = ctx.enter_context(tc.tile_pool(name="o", bufs=2))
        ps_s0 = ctx.enter_context(tc.tile_pool(name="ps_s0", bufs=2, space="PSUM"))
        ps_mid = ctx.enter_context(tc.tile_pool(name="ps_mid", bufs=2, space="PSUM"))
        ps_mlp = ctx.enter_context(tc.tile_pool(name="ps_mlp", bufs=1, space="PSUM"))

        for t in range(n_tiles):
            sl = ts(t, free)

            # Arep_a[p, n] = aT[a, n] for all p — host-replicated, plain DMA.
            arep = arep_pool.tile([M0, A * free], DBF16)
            for a in range(A):
                nc.sync.dma_start(
                    arep[:, ts(a, free)], atrep[ts(a, M0), sl]
                )

            # s0T accumulation over the 16 a-chunks of c=(a,i).
            s0_ps = ps_s0.tile([M0, free], F32)
            for a in range(A):
                u0 = u_pool.tile([M0, free], DBF16, tag="u0")
                nc.vector.tensor_mul(u0[:], x0t_sb[:, sl], arep[:, ts(a, free)])
                nc.tensor.matmul(
                    s0_ps[:], w0_sb[:, ts(a, M0)], u0[:],
                    start=(a == 0), stop=(a == A - 1),
                )

            # silu(s0 + b1) = (s0+b1) * sigmoid(s0+b1); CoreSim has no Silu LUT.
            s_pre = s_pool.tile([M0, free], DBF16, tag="s_pre")
            nc.scalar.activation(s_pre[:], s0_ps[:], AF.Identity, bias=b1_sb[:])
            s_sig = s_pool.tile([M0, free], DBF16, tag="s_sig")
            nc.scalar.activation(s_sig[:], s0_ps[:], AF.Sigmoid, bias=b1_sb[:])
            scal = s_pool.tile([M0, free], DBF16, tag="scal")
            nc.vector.tensor_mul(scal[:], s_pre[:], s_sig[:])

            # midT accumulation.
            mid_ps = ps_mid.tile([A, free], F32)
            for a in range(A):
                u3 = u_pool.tile([M0, free], DBF16, tag="u3")
                # split the multiplies across DVE and the idle GPSIMD
                eng = nc.vector if a % 2 == 0 else nc.gpsimd
                eng.tensor_mul(u3[:], scal[:], arep[:, ts(a, free)])
                nc.tensor.matmul(
                    mid_ps[:], w2_sb[:, ts(a, A)], u3[:],
                    start=(a == 0), stop=(a == A - 1),
                )

            midb = s_pool.tile([A, free], DBF16, tag="midb")
            nc.scalar.activation(midb[:], mid_ps[:], AF.Identity, bias=b2_sb[:])

            h_ps = ps_mlp.tile([A, free], F32, tag="h")
            nc.tensor.matmul(h_ps[:], w3_sb[:], midb[:], start=True, stop=True)
            h_pre = s_pool.tile([A, free], DBF16, tag="h_pre")
            nc.scalar.activation(h_pre[:], h_ps[:], AF.Identity, bias=b3_sb[:])
            h_sig = s_pool.tile([A, free], DBF16, tag="h_sig")
            nc.scalar.activation(h_sig[:], h_ps[:], AF.Sigmoid, bias=b3_sb[:])
            hb = s_pool.tile([A, free], DBF16, tag="hb")
            nc.vector.tensor_mul(hb[:], h_pre[:], h_sig[:])

            out_ps = ps_mlp.tile([1, free], F32, tag="out")
            nc.tensor.matmul(out_ps[:], w4_sb[:], hb[:], start=True, stop=True)
            ob = o_pool.tile([1, free], F32)
            nc.scalar.activation(ob[:], out_ps[:], AF.Identity, bias=b4_sb[:])
            nc.sync.dma_start(outt[:, sl], ob[:])

    nc.compile()
    return nc


def prep_host(inputs: dict, nsh: int = NSH, n_cores: int = N_CORES):
    """Host-side prep: slice/transpose/cast inputs, build per-core in_maps."""
    node_vec = np.asarray(inputs["node_vec"], dtype=np.float32)
    node_embedding = np.asarray(inputs["node_embedding"], dtype=np.float32)
    W1_l0 = np.asarray(inputs["W1_l0"], dtype=np.float32)
    b1 = np.asarray(inputs["b1"], dtype=np.float32)
    W2 = np.asarray(inputs["W2"], dtype=np.float32)
    b2 = np.asarray(inputs["b2"], dtype=np.float32)
    W3 = np.asarray(inputs["W3"], dtype=np.float32)
    b3 = np.asarray(inputs["b3"], dtype=np.float32)
    W4 = np.asarray(inputs["W4"], dtype=np.float32)
    b4 = np.asarray(inputs["b4"], dtype=np.float32)

    x0T = np.ascontiguousarray(node_vec[:, :M0].T).astype(BF16)      # [128, N]
    aT = np.ascontiguousarray(node_embedding.T).astype(BF16)         # [16, N]
    aTrep = np.ascontiguousarray(np.repeat(aT, M0, axis=0))          # [2048, N]

    w0h = (W1_l0[:, :, :M0] * SCALE).reshape(M0, A * M0).astype(BF16)
    w2h = (W2 * SCALE).reshape(M0, A * A).astype(BF16)
    w3h = (W3 / np.sqrt(A)).astype(BF16)
    w4h = (W4 / np.sqrt(A)).astype(BF16)

    shared = {
        "w0": w0h, "w2": w2h, "w3": w3h, "w4": w4h,
        "b1": np.ascontiguousarray(b1[:M0].reshape(M0, 1)),
        "b2": np.ascontiguousarray(b2.reshape(A, 1)),
        "b3": np.ascontiguousarray(b3.reshape(A, 1)),
        "b4": np.ascontiguousarray(b4.reshape(1, 1)),
    }
    in_maps = []
    for c in range(n_cores):
        sl = slice(c * nsh, (c + 1) * nsh)
        in_maps.append({
            "x0t": np.ascontiguousarray(x0T[:, sl]),
            "atrep": np.ascontiguousarray(aTrep[:, sl]),
            **shared,
        })
    return in_maps


_NC_CACHE = {}


def _get_nc():
    if "nc" not in _NC_CACHE:
        _NC_CACHE["nc"] = build_nc()
    return _NC_CACHE["nc"]


def kernel_with_results(trace: bool = False, **inputs):
    nc = _get_nc()
    in_maps = prep_host(inputs)
    res = run_bass_kernel_spmd(
        nc, in_maps, core_ids=list(range(N_CORES)), trace=trace,
    )
    out = np.empty((N_FULL, 1), dtype=np.float32)
    for c in range(N_CORES):
        out[c * NSH:(c + 1) * NSH, 0] = res.results[c]["outt"][0]
    return out, res


def kernel(**inputs) -> np.ndarray:
    out, _ = kernel_with_results(trace=False, **inputs)
    return out



# revision 7
# speedup vs baseline: 1.6404x; 1.6404x over previous
"""Trainium2 Bass kernel for nn_InvariantHeadviaTP.

Reference computation (after dead-code elimination -- y1/y2/gates are never
used by the output):
    x0   = node_vec[:, :128]                  # [N, 128]
    a    = node_embedding                     # [N, 16]
    s0   = einsum('ni,na,iak->nk', x0, a, W1_l0[:, :, :128]) / sqrt(2048) + b1[:128]
    scal = silu(s0)                           # [N, 128]
    mid  = einsum('ni,na,iak->nk', scal, a, W2) / sqrt(2048) + b2   # [N, 16]
    h    = silu(mid @ W3 / 4 + b3)            # [N, 16]
    out  = h @ W4 / 4 + b4                    # [N, 1]

Strategy: data-parallel over 8 cores (2048 nodes each), transposed layout
(features on SBUF partitions, nodes on the free dim).

s0 contraction over c=(i,a) [128*16=2048] is chunked into 16 K=128 chunks
of (32 i's x 4 a's): chunk (q, c4) covers i in [32q, 32q+32), a in
[4*c4, 4*c4+4).  The moving operand for a chunk is
    u[p, n] = x0rep_q[p, n] * a4rep_c4[p, n]
      x0rep_q[p, n] = x0T[32q + p%32, n]   (x0 rows tiled 4x -> 2MB/core)
      a4rep_c4[p,n] = aT[4c4 + p//32, n]   (a rows repeated 32x -> 2MB/core)
built by one [128, 1024] DVE multiply per chunk per node-half (4x DVE mode).
This halves the broadcast DMA vs replicating aT across all 128 partitions
(8MB/core) while keeping all operands SBUF/bf16/contiguous.

mid path avoids 16 M=16 matmuls: P[(a,k), n] = sum_i W2[i,a,k] scal[i,n]
via 2 M=128 matmuls, gate-multiply by a16rep (a rows repeated 16x), then a
one-hot selector matmul sums the 8 a's per half: 8192 PE columns instead
of 32768.
"""

import numpy as np
import ml_dtypes
from contextlib import ExitStack

import concourse.bass as bass
import concourse.mybir as mybir
import concourse.tile as tile
from concourse import bacc
from concourse.bass import ts
from concourse.bass_utils import run_bass_kernel_spmd

N_CORES = 8
N_FULL = 16384
NSH = N_FULL // N_CORES          # 2048 nodes per core
A = 16                           # attr dim
M0 = 128                         # MUL0 (scalar channels)
HALF = 1024                      # node half per core
FT = 512                         # matmul moving free size (PSUM bank)
SCALE = 1.0 / np.sqrt(M0 * A)    # path normalization of both fctp einsums
BF16 = ml_dtypes.bfloat16

AF = mybir.ActivationFunctionType
F32 = mybir.dt.float32
DBF16 = mybir.dt.bfloat16


def build_nc(nsh: int = NSH, num_devices: int = N_CORES):
    assert nsh % HALF == 0
    n_halves = nsh // HALF

    nc = bacc.Bacc(
        "TRN2",
        target_bir_lowering=False,
        debug=False,
        enable_asserts=False,
        num_devices=num_devices,
    )

    # node data (per-core shard, host-prepped layouts)
    x0rep = nc.dram_tensor("x0rep", [M0, 4 * nsh], DBF16, kind="ExternalInput").ap()
    a4rep = nc.dram_tensor("a4rep", [M0, 4 * nsh], DBF16, kind="ExternalInput").ap()
    a16rep = nc.dram_tensor("a16rep", [M0, 2 * nsh], DBF16, kind="ExternalInput").ap()
    # weights
    w0 = nc.dram_tensor("w0", [M0, 16 * M0], DBF16, kind="ExternalInput").ap()
    w2p = nc.dram_tensor("w2p", [M0, 2 * M0], DBF16, kind="ExternalInput").ap()
    selh = nc.dram_tensor("selh", [M0, A], DBF16, kind="ExternalInput").ap()
    w3 = nc.dram_tensor("w3", [A, A], DBF16, kind="ExternalInput").ap()
    w4 = nc.dram_tensor("w4", [A, 1], DBF16, kind="ExternalInput").ap()
    b1 = nc.dram_tensor("b1", [M0, 1], F32, kind="ExternalInput").ap()
    b2 = nc.dram_tensor("b2", [A, 1], F32, kind="ExternalInput").ap()
    b3 = nc.dram_tensor("b3", [A, 1], F32, kind="ExternalInput").ap()
    b4 = nc.dram_tensor("b4", [1, 1], F32, kind="ExternalInput").ap()
    outt = nc.dram_tensor("outt", [1, nsh], F32, kind="ExternalOutput").ap()

    with tile.TileContext(nc) as tc, ExitStack() as ctx:
        consts = ctx.enter_context(tc.tile_pool(name="consts", bufs=1))

        w0_sb = consts.tile([M0, 16 * M0], DBF16)
        nc.sync.dma_start(w0_sb[:], w0)
        w2p_sb = consts.tile([M0, 2 * M0], DBF16)
        nc.sync.dma_start(w2p_sb[:], w2p)
        selh_sb = consts.tile([M0, A], DBF16)
        nc.sync.dma_start(selh_sb[:], selh)
        w3_sb = consts.tile([A, A], DBF16)
        nc.sync.dma_start(w3_sb[:], w3)
        w4_sb = consts.tile([A, 1], DBF16)
        nc.sync.dma_start(w4_sb[:], w4)
        b1_sb = consts.tile([M0, 1], F32)
        nc.sync.dma_start(b1_sb[:], b1)
        b2_sb = consts.tile([A, 1], F32)
        nc.sync.dma_start(b2_sb[:], b2)
        b3_sb = consts.tile([A, 1], F32)
        nc.sync.dma_start(b3_sb[:], b3)
        b4_sb = consts.tile([1, 1], F32)
        nc.sync.dma_start(b4_sb[:], b4)

        # node-data SBUF residents: loaded block-wise so compute can start
        # as soon as the first blocks land.
        x0rep_sb = consts.tile([M0, 4 * nsh], DBF16)
        a4rep_sb = consts.tile([M0, 4 * nsh], DBF16)
        a16rep_sb = consts.tile([M0, 2 * nsh], DBF16)
        nc.sync.dma_start(x0rep_sb[:, ts(0, nsh)], x0rep[:, ts(0, nsh)])
        nc.sync.dma_start(a4rep_sb[:, ts(0, nsh)], a4rep[:, ts(0, nsh)])
        for blk in range(1, 4):
            nc.sync.dma_start(a4rep_sb[:, ts(blk, nsh)], a4rep[:, ts(blk, nsh)])
        for blk in range(1, 4):
            nc.sync.dma_start(x0rep_sb[:, ts(blk, nsh)], x0rep[:, ts(blk, nsh)])
        for blk in range(2):
            nc.sync.dma_start(a16rep_sb[:, ts(blk, nsh)], a16rep[:, ts(blk, nsh)])

        u_pool = ctx.enter_context(tc.tile_pool(name="u", bufs=3))
        s_pool = ctx.enter_context(tc.tile_pool(name="s", bufs=2))
        scal_pool = ctx.enter_context(tc.tile_pool(name="scal", bufs=4))
        pm_pool = ctx.enter_context(tc.tile_pool(name="pm", bufs=2))
        o_pool = ctx.enter_context(tc.tile_pool(name="o", bufs=2))
        ps_s0 = ctx.enter_context(tc.tile_pool(name="ps_s0", bufs=2, space="PSUM"))
        ps_p = ctx.enter_context(tc.tile_pool(name="ps_p", bufs=1, space="PSUM"))
        ps_mm = ctx.enter_context(tc.tile_pool(name="ps_mm", bufs=2, space="PSUM"))

        for h in range(n_halves):
            hs = ts(h, HALF)  # node slice of this half within the shard

            # ---- s0 accumulation: 16 chunks of K=128 = (32 i x 4 a) ----
            s0_ps = [
                ps_s0.tile([M0, FT], F32, tag=f"s0_{f}", name=f"s0_{h}_{f}")
                for f in range(2)
            ]
            for ci in range(16):
                q, c4 = ci // 4, ci % 4
                u = u_pool.tile([M0, HALF], DBF16, tag="u")
                nc.vector.tensor_mul(
                    u[:],
                    x0rep_sb[:, bass.ds(q * nsh + h * HALF, HALF)],
                    a4rep_sb[:, bass.ds(c4 * nsh + h * HALF, HALF)],
                )
                for f in range(2):
                    nc.tensor.matmul(
                        s0_ps[f][:],
                        w0_sb[:, ts(ci, M0)],
                        u[:, ts(f, FT)],
                        start=(ci == 0),
                        stop=(ci == 15),
                    )

            # ---- epilogue per 512-node tile ----
            for f in range(2):
                nsl = ts(2 * h + f, FT)  # node slice within the shard

                s_sig = s_pool.tile([M0, FT], DBF16, tag="s_sig")
                nc.scalar.activation(s_sig[:], s0_ps[f][:], AF.Sigmoid, bias=b1_sb[:])
                s_idn = s_pool.tile([M0, FT], DBF16, tag="s_idn")
                nc.scalar.activation(s_idn[:], s0_ps[f][:], AF.Identity, bias=b1_sb[:])
                scal = scal_pool.tile([M0, FT], DBF16, tag="scal")
                nc.vector.tensor_mul(scal[:], s_idn[:], s_sig[:])

                # P[(a_l,k), n] for a-halves 0/1, gate by a16rep, then
                # selector-sum the 8 a's of each half into mid.
                mm = ps_mm.tile([65, FT], F32, tag="mm")
                mid_ps = mm[0:16, :]
                h_ps = mm[32:48, :]
                out_ps = mm[64:65, :]

                pm_t = []
                for ah in range(2):
                    p_ps = ps_p.tile([M0, FT], F32, tag=f"p_{ah}")
                    nc.tensor.matmul(
                        p_ps[:], w2p_sb[:, ts(ah, M0)], scal[:],
                        start=True, stop=True,
                    )
                    pm = pm_pool.tile([M0, FT], DBF16, tag=f"pm_{ah}")
                    nc.vector.tensor_mul(
                        pm[:], p_ps[:],
                        a16rep_sb[:, bass.ds(ah * nsh + (2 * h + f) * FT, FT)],
                    )
                    pm_t.append(pm)
                for ah in range(2):
                    nc.tensor.matmul(
                        mid_ps[:], selh_sb[:], pm_t[ah][:],
                        start=(ah == 0), stop=(ah == 1),
                    )

                midb = s_pool.tile([A, FT], DBF16, tag="midb")
                nc.scalar.activation(midb[:], mid_ps[:], AF.Identity, bias=b2_sb[:])

                nc.tensor.matmul(h_ps[:], w3_sb[:], midb[:], start=True, stop=True)
                h_sig = s_pool.tile([A, FT], DBF16, tag="h_sig")
                nc.scalar.activation(h_sig[:], h_ps[:], AF.Sigmoid, bias=b3_sb[:])
                h_idn = s_pool.tile([A, FT], DBF16, tag="h_idn")
                nc.scalar.activation(h_idn[:], h_ps[:], AF.Identity, bias=b3_sb[:])
                hb = s_pool.tile([A, FT], DBF16, tag="hb")
                nc.vector.tensor_mul(hb[:], h_idn[:], h_sig[:])

                nc.tensor.matmul(out_ps[:], w4_sb[:], hb[:], start=True, stop=True)
                ob = o_pool.tile([1, FT], F32)
                nc.scalar.activation(ob[:], out_ps[:], AF.Identity, bias=b4_sb[:])
                nc.sync.dma_start(outt[:, nsl], ob[:])

    nc.compile()
    return nc


def prep_host(inputs: dict, nsh: int = NSH, n_cores: int = N_CORES):
    """Host-side prep: slice/transpose/cast inputs, build per-core in_maps."""
    node_vec = np.asarray(inputs["node_vec"], dtype=np.float32)
    node_embedding = np.asarray(inputs["node_embedding"], dtype=np.float32)
    W1_l0 = np.asarray(inputs["W1_l0"], dtype=np.float32)
    b1 = np.asarray(inputs["b1"], dtype=np.float32)
    W2 = np.asarray(inputs["W2"], dtype=np.float32)
    b2 = np.asarray(inputs["b2"], dtype=np.float32)
    W3 = np.asarray(inputs["W3"], dtype=np.float32)
    b3 = np.asarray(inputs["b3"], dtype=np.float32)
    W4 = np.asarray(inputs["W4"], dtype=np.float32)
    b4 = np.asarray(inputs["b4"], dtype=np.float32)

    x0T = np.ascontiguousarray(node_vec[:, :M0].T).astype(BF16)      # [128, N]
    aT = np.ascontiguousarray(node_embedding.T).astype(BF16)         # [16, N]

    # w0 chunk ci = (q, c4): [p, k] = W[32q + p%32, 4c4 + p//32, k]
    W = (W1_l0[:, :, :M0] * SCALE).astype(np.float32)                # [128,16,128]
    w0_blocks = []
    for ci in range(16):
        q, c4 = ci // 4, ci % 4
        blk = W[q * 32:(q + 1) * 32, c4 * 4:(c4 + 1) * 4, :]         # [32, 4, 128]
        w0_blocks.append(blk.transpose(1, 0, 2).reshape(M0, M0))     # p = a_l*32+i_l
    w0h = np.concatenate(w0_blocks, axis=1).astype(BF16)             # [128, 2048]

    w2ph = (W2 * SCALE).reshape(M0, A * A).astype(BF16)              # [128, 256]
    selh = np.tile(np.eye(A, dtype=np.float32), (8, 1)).astype(BF16)  # [128, 16]
    w3h = (W3 / np.sqrt(A)).astype(BF16)
    w4h = (W4 / np.sqrt(A)).astype(BF16)

    shared = {
        "w0": w0h, "w2p": w2ph, "selh": selh, "w3": w3h, "w4": w4h,
        "b1": np.ascontiguousarray(b1[:M0].reshape(M0, 1)),
        "b2": np.ascontiguousarray(b2.reshape(A, 1)),
        "b3": np.ascontiguousarray(b3.reshape(A, 1)),
        "b4": np.ascontiguousarray(b4.reshape(1, 1)),
    }
    in_maps = []
    for c in range(n_cores):
        sl = slice(c * nsh, (c + 1) * nsh)
        x0s = x0T[:, sl]
        ats = aT[:, sl]
        x0rep = np.concatenate(
            [np.tile(x0s[q * 32:(q + 1) * 32, :], (4, 1)) for q in range(4)], axis=1
        )                                                            # [128, 4*nsh]
        a4rep = np.concatenate(
            [np.repeat(ats[c4 * 4:(c4 + 1) * 4, :], 32, axis=0) for c4 in range(4)],
            axis=1,
        )                                                            # [128, 4*nsh]
        a16rep = np.concatenate(
            [np.repeat(ats[ah * 8:(ah + 1) * 8, :], 16, axis=0) for ah in range(2)],
            axis=1,
        )                                                            # [128, 2*nsh]
        in_maps.append({
            "x0rep": np.ascontiguousarray(x0rep),
            "a4rep": np.ascontiguousarray(a4rep),
            "a16rep": np.ascontiguousarray(a16rep),
            **shared,
        })
    return in_maps


_NC_CACHE = {}


def _get_nc():
    if "nc" not in _NC_CACHE:
        _NC_CACHE["nc"] = build_nc()
    return _NC_CACHE["nc"]


def kernel_with_results(trace: bool = False, **inputs):
    nc = _get_nc()
    in_maps = prep_host(inputs)
    res = run_bass_kernel_spmd(
        nc, in_maps, core_ids=list(range(N_CORES)), trace=trace,
    )
    out = np.empty((N_FULL, 1), dtype=np.float32)
    for c in range(N_CORES):
        out[c * NSH:(c + 1) * NSH, 0] = res.results[c]["outt"][0]
    return out, res


def kernel(**inputs) -> np.ndarray:
    out, _ = kernel_with_results(trace=False, **inputs)
    return out


# revision 11
# speedup vs baseline: 1.7411x; 1.0614x over previous
"""Trainium2 Bass kernel for nn_InvariantHeadviaTP.

Reference computation (after dead-code elimination -- y1/y2/gates are never
used by the output):
    x0   = node_vec[:, :128]                  # [N, 128]
    a    = node_embedding                     # [N, 16]
    s0   = einsum('ni,na,iak->nk', x0, a, W1_l0[:, :, :128]) / sqrt(2048) + b1[:128]
    scal = silu(s0)                           # [N, 128]
    mid  = einsum('ni,na,iak->nk', scal, a, W2) / sqrt(2048) + b2   # [N, 16]
    h    = silu(mid @ W3 / 4 + b3)            # [N, 16]
    out  = h @ W4 / 4 + b4                    # [N, 1]

Strategy: data-parallel over 8 cores (2048 nodes each), transposed layout
(features on SBUF partitions, nodes on the free dim).

s0 contraction over c=(i,a) [128*16=2048] is chunked into 16 K=128 chunks
of (32 i's x 4 a's): chunk (q, c4) covers i in [32q, 32q+32), a in
[4*c4, 4*c4+4).  The moving operand for a chunk is
    u[p, n] = x0rep_q[p, n] * a4rep_c4[p, n]
      x0rep_q[p, n] = x0T[32q + p%32, n]   (x0 rows tiled 4x -> 2MB/core)
      a4rep_c4[p,n] = aT[4c4 + p//32, n]   (a rows repeated 32x -> 2MB/core)
built by one [128, 1024] DVE multiply per chunk per node-half (4x DVE mode).
This halves the broadcast DMA vs replicating aT across all 128 partitions
(8MB/core) while keeping all operands SBUF/bf16/contiguous.

mid path avoids 16 M=16 matmuls: P[(a,k), n] = sum_i W2[i,a,k] scal[i,n]
via 2 M=128 matmuls, gate-multiply by a16rep (a rows repeated 16x), then a
one-hot selector matmul sums the 8 a's per half: 8192 PE columns instead
of 32768.
"""

import numpy as np
import ml_dtypes
from contextlib import ExitStack

import concourse.bass as bass
import concourse.mybir as mybir
import concourse.tile as tile
from concourse import bacc
from concourse.bass import ts
from concourse.bass_utils import run_bass_kernel_spmd

N_CORES = 8
N_FULL = 16384
NSH = N_FULL // N_CORES          # 2048 nodes per core
A = 16                           # attr dim
M0 = 128                         # MUL0 (scalar channels)
HALF = 1024                      # node half per core
FT = 512                         # matmul moving free size (PSUM bank)
SCALE = 1.0 / np.sqrt(M0 * A)    # path normalization of both fctp einsums
BF16 = ml_dtypes.bfloat16

AF = mybir.ActivationFunctionType
F32 = mybir.dt.float32
DBF16 = mybir.dt.bfloat16


def build_nc(nsh: int = NSH, num_devices: int = N_CORES):
    assert nsh == 4 * FT, "kernel is laid out for 4 PSUM-bank node tiles"

    nc = bacc.Bacc(
        "TRN2",
        target_bir_lowering=False,
        debug=False,
        enable_asserts=False,
        num_devices=num_devices,
    )

    # node data (per-core shard, host-prepped layouts)
    x0rep = nc.dram_tensor("x0rep", [M0, 4 * nsh], DBF16, kind="ExternalInput").ap()
    a4rep = nc.dram_tensor("a4rep", [M0, 4 * nsh], DBF16, kind="ExternalInput").ap()
    a16rep = nc.dram_tensor("a16rep", [M0, 2 * nsh], DBF16, kind="ExternalInput").ap()
    # weights
    w0 = nc.dram_tensor("w0", [M0, 16 * M0], DBF16, kind="ExternalInput").ap()
    w2p = nc.dram_tensor("w2p", [M0, 2 * M0], DBF16, kind="ExternalInput").ap()
    selh = nc.dram_tensor("selh", [M0, A], DBF16, kind="ExternalInput").ap()
    w3 = nc.dram_tensor("w3", [A, A], DBF16, kind="ExternalInput").ap()
    w4 = nc.dram_tensor("w4", [A, 1], DBF16, kind="ExternalInput").ap()
    b1 = nc.dram_tensor("b1", [M0, 1], F32, kind="ExternalInput").ap()
    b2 = nc.dram_tensor("b2", [A, 1], F32, kind="ExternalInput").ap()
    b3 = nc.dram_tensor("b3", [A, 1], F32, kind="ExternalInput").ap()
    b4 = nc.dram_tensor("b4", [1, 1], F32, kind="ExternalInput").ap()
    outt = nc.dram_tensor("outt", [1, nsh], F32, kind="ExternalOutput").ap()

    with tile.TileContext(nc) as tc, ExitStack() as ctx:
        consts = ctx.enter_context(tc.tile_pool(name="consts", bufs=1))

        # w0 first: the very first matmul needs it.
        w0_sb = consts.tile([M0, 16 * M0], DBF16)
        nc.sync.dma_start(w0_sb[:], w0)
        w2p_sb = consts.tile([M0, 2 * M0], DBF16)
        nc.sync.dma_start(w2p_sb[:], w2p)
        selh_sb = consts.tile([M0, A], DBF16)
        nc.sync.dma_start(selh_sb[:], selh)
        w3_sb = consts.tile([A, A], DBF16)
        nc.sync.dma_start(w3_sb[:], w3)
        w4_sb = consts.tile([A, 1], DBF16)
        nc.sync.dma_start(w4_sb[:], w4)
        b1_sb = consts.tile([M0, 1], F32)
        nc.sync.dma_start(b1_sb[:], b1)
        b2_sb = consts.tile([A, 1], F32)
        nc.sync.dma_start(b2_sb[:], b2)
        b3_sb = consts.tile([A, 1], F32)
        nc.sync.dma_start(b3_sb[:], b3)
        b4_sb = consts.tile([1, 1], F32)
        nc.sync.dma_start(b4_sb[:], b4)

        # Node-data residents: one tile + one DMA per block so each chunk's
        # compute only waits for its own blocks. Issued from the otherwise
        # idle gpsimd queue (SP serializes DMA issue at ~600ns each).
        x0rep_sb = []
        a4rep_sb = []
        a16rep_sb = []
        for blk in range(4):
            t = consts.tile([M0, nsh], DBF16, name=f"x0rep_sb{blk}")
            x0rep_sb.append(t)
        for blk in range(4):
            t = consts.tile([M0, nsh], DBF16, name=f"a4rep_sb{blk}")
            a4rep_sb.append(t)
        for blk in range(2):
            t = consts.tile([M0, nsh], DBF16, name=f"a16rep_sb{blk}")
            a16rep_sb.append(t)
        # issue in first-use order: chunk ci=(q,c4) uses x0[q], a4[c4]
        nc.gpsimd.dma_start(x0rep_sb[0][:], x0rep[:, ts(0, nsh)])
        for blk in range(4):
            nc.gpsimd.dma_start(a4rep_sb[blk][:], a4rep[:, ts(blk, nsh)])
        for blk in range(1, 4):
            nc.gpsimd.dma_start(x0rep_sb[blk][:], x0rep[:, ts(blk, nsh)])
        for blk in range(2):
            nc.gpsimd.dma_start(a16rep_sb[blk][:], a16rep[:, ts(blk, nsh)])

        u_pool = ctx.enter_context(tc.tile_pool(name="u", bufs=3))
        s_pool = ctx.enter_context(tc.tile_pool(name="s", bufs=2))
        scal_pool = ctx.enter_context(tc.tile_pool(name="scal", bufs=4))
        pm_pool = ctx.enter_context(tc.tile_pool(name="pm", bufs=4))
        o_pool = ctx.enter_context(tc.tile_pool(name="o", bufs=2))
        # PSUM budget (8 banks): s0 4 tags + p 2 tags + mm 2 tags, bufs=1.
        ps_s0 = ctx.enter_context(tc.tile_pool(name="ps_s0", bufs=1, space="PSUM"))
        ps_p = ctx.enter_context(tc.tile_pool(name="ps_p", bufs=1, space="PSUM"))
        ps_mm = ctx.enter_context(tc.tile_pool(name="ps_mm", bufs=1, space="PSUM"))

        # ---- s0 accumulation: 16 chunks of K=128 = (32 i x 4 a), full
        # shard width per chunk; w0 lhsT loaded once per chunk. ----
        s0_ps = [
            ps_s0.tile([M0, FT], F32, tag=f"s0_{f}", name=f"s0_{f}")
            for f in range(4)
        ]
        for ci in range(16):
            q, c4 = ci // 4, ci % 4
            u = u_pool.tile([M0, nsh], DBF16, tag="u")
            nc.vector.tensor_mul(u[:], x0rep_sb[q][:], a4rep_sb[c4][:])
            for f in range(4):
                nc.tensor.matmul(
                    s0_ps[f][:],
                    w0_sb[:, ts(ci, M0)],
                    u[:, ts(f, FT)],
                    start=(ci == 0),
                    stop=(ci == 15),
                )

        # ---- epilogue in pairs of 512-node tiles; matmuls grouped by
        # lhsT so each weight is loaded once per pair. ----
        for g in range(2):
            fs = [2 * g, 2 * g + 1]

            scal = {}
            for f in fs:
                s_sig = s_pool.tile([M0, FT], DBF16, tag="s_sig")
                nc.scalar.activation(s_sig[:], s0_ps[f][:], AF.Sigmoid, bias=b1_sb[:])
                s_idn = s_pool.tile([M0, FT], DBF16, tag="s_idn")
                nc.scalar.activation(s_idn[:], s0_ps[f][:], AF.Identity, bias=b1_sb[:])
                sc = scal_pool.tile([M0, FT], DBF16, tag=f"scal_{f % 2}",
                                    name=f"scal_{f}")
                nc.vector.tensor_mul(sc[:], s_idn[:], s_sig[:])
                scal[f] = sc

            # P[(a_l,k), n] for a-halves 0/1, gated by a16rep; selector-sum
            # the 8 a's of each half into mid.
            mm = {}
            for f in fs:
                mm[f] = ps_mm.tile([65, FT], F32, tag=f"mm_{f % 2}", name=f"mm_{f}")
            pm_t = {}
            for ah in range(2):
                for f in fs:
                    p_ps = ps_p.tile([M0, FT], F32, tag=f"p_{f % 2}",
                                     name=f"p_{ah}_{f}")
                    nc.tensor.matmul(
                        p_ps[:], w2p_sb[:, ts(ah, M0)], scal[f][:],
                        start=True, stop=True,
                    )
                    pm = pm_pool.tile([M0, FT], DBF16, tag=f"pm_{ah}_{f % 2}",
                                      name=f"pm_{ah}_{f}")
                    nc.vector.tensor_mul(
                        pm[:], p_ps[:],
                        a16rep_sb[ah][:, ts(f, FT)],
                    )
                    pm_t[(ah, f)] = pm
            for f in fs:
                for ah in range(2):
                    nc.tensor.matmul(
                        mm[f][0:16, :], selh_sb[:], pm_t[(ah, f)][:],
                        start=(ah == 0), stop=(ah == 1),
                    )

            midb = {}
            for f in fs:
                mb = s_pool.tile([A, FT], DBF16, tag="midb", name=f"midb_{f}")
                nc.scalar.activation(mb[:], mm[f][0:16, :], AF.Identity, bias=b2_sb[:])
                midb[f] = mb
            for f in fs:
                nc.tensor.matmul(mm[f][32:48, :], w3_sb[:], midb[f][:],
                                 start=True, stop=True)
            hb = {}
            for f in fs:
                h_sig = s_pool.tile([A, FT], DBF16, tag="h_sig")
                nc.scalar.activation(h_sig[:], mm[f][32:48, :], AF.Sigmoid,
                                     bias=b3_sb[:])
                h_idn = s_pool.tile([A, FT], DBF16, tag="h_idn")
                nc.scalar.activation(h_idn[:], mm[f][32:48, :], AF.Identity,
                                     bias=b3_sb[:])
                hbt = s_pool.tile([A, FT], DBF16, tag="hb", name=f"hb_{f}")
                nc.vector.tensor_mul(hbt[:], h_idn[:], h_sig[:])
                hb[f] = hbt
            for f in fs:
                nc.tensor.matmul(mm[f][64:65, :], w4_sb[:], hb[f][:],
                                 start=True, stop=True)
            for f in fs:
                ob = o_pool.tile([1, FT], F32, tag="ob", name=f"ob_{f}")
                nc.scalar.activation(ob[:], mm[f][64:65, :], AF.Identity,
                                     bias=b4_sb[:])
                nc.sync.dma_start(outt[:, ts(f, FT)], ob[:])

    nc.compile()
    return nc


def prep_host(inputs: dict, nsh: int = NSH, n_cores: int = N_CORES):
    """Host-side prep: slice/transpose/cast inputs, build per-core in_maps."""
    node_vec = np.asarray(inputs["node_vec"], dtype=np.float32)
    node_embedding = np.asarray(inputs["node_embedding"], dtype=np.float32)
    W1_l0 = np.asarray(inputs["W1_l0"], dtype=np.float32)
    b1 = np.asarray(inputs["b1"], dtype=np.float32)
    W2 = np.asarray(inputs["W2"], dtype=np.float32)
    b2 = np.asarray(inputs["b2"], dtype=np.float32)
    W3 = np.asarray(inputs["W3"], dtype=np.float32)
    b3 = np.asarray(inputs["b3"], dtype=np.float32)
    W4 = np.asarray(inputs["W4"], dtype=np.float32)
    b4 = np.asarray(inputs["b4"], dtype=np.float32)

    x0T = np.ascontiguousarray(node_vec[:, :M0].T).astype(BF16)      # [128, N]
    aT = np.ascontiguousarray(node_embedding.T).astype(BF16)         # [16, N]

    # w0 chunk ci = (q, c4): [p, k] = W[32q + p%32, 4c4 + p//32, k]
    W = (W1_l0[:, :, :M0] * SCALE).astype(np.float32)                # [128,16,128]
    w0_blocks = []
    for ci in range(16):
        q, c4 = ci // 4, ci % 4
        blk = W[q * 32:(q + 1) * 32, c4 * 4:(c4 + 1) * 4, :]         # [32, 4, 128]
        w0_blocks.append(blk.transpose(1, 0, 2).reshape(M0, M0))     # p = a_l*32+i_l
    w0h = np.concatenate(w0_blocks, axis=1).astype(BF16)             # [128, 2048]

    w2ph = (W2 * SCALE).reshape(M0, A * A).astype(BF16)              # [128, 256]
    selh = np.tile(np.eye(A, dtype=np.float32), (8, 1)).astype(BF16)  # [128, 16]
    w3h = (W3 / np.sqrt(A)).astype(BF16)
    w4h = (W4 / np.sqrt(A)).astype(BF16)

    shared = {
        "w0": w0h, "w2p": w2ph, "selh": selh, "w3": w3h, "w4": w4h,
        "b1": np.ascontiguousarray(b1[:M0].reshape(M0, 1)),
        "b2": np.ascontiguousarray(b2.reshape(A, 1)),
        "b3": np.ascontiguousarray(b3.reshape(A, 1)),
        "b4": np.ascontiguousarray(b4.reshape(1, 1)),
    }
    in_maps = []
    for c in range(n_cores):
        sl = slice(c * nsh, (c + 1) * nsh)
        x0s = x0T[:, sl]
        ats = aT[:, sl]
        x0rep = np.concatenate(
            [np.tile(x0s[q * 32:(q + 1) * 32, :], (4, 1)) for q in range(4)], axis=1
        )                                                            # [128, 4*nsh]
        a4rep = np.concatenate(
            [np.repeat(ats[c4 * 4:(c4 + 1) * 4, :], 32, axis=0) for c4 in range(4)],
            axis=1,
        )                                                            # [128, 4*nsh]
        a16rep = np.concatenate(
            [np.repeat(ats[ah * 8:(ah + 1) * 8, :], 16, axis=0) for ah in range(2)],
            axis=1,
        )                                                            # [128, 2*nsh]
        in_maps.append({
            "x0rep": np.ascontiguousarray(x0rep),
            "a4rep": np.ascontiguousarray(a4rep),
            "a16rep": np.ascontiguousarray(a16rep),
            **shared,
        })
    return in_maps


_NC_CACHE = {}


def _get_nc():
    if "nc" not in _NC_CACHE:
        _NC_CACHE["nc"] = build_nc()
    return _NC_CACHE["nc"]


def kernel_with_results(trace: bool = False, **inputs):
    nc = _get_nc()
    in_maps = prep_host(inputs)
    res = run_bass_kernel_spmd(
        nc, in_maps, core_ids=list(range(N_CORES)), trace=trace,
    )
    out = np.empty((N_FULL, 1), dtype=np.float32)
    for c in range(N_CORES):
        out[c * NSH:(c + 1) * NSH, 0] = res.results[c]["outt"][0]
    return out, res


def kernel(**inputs) -> np.ndarray:
    out, _ = kernel_with_results(trace=False, **inputs)
    return out


# revision 17
# speedup vs baseline: 1.9749x; 1.1343x over previous
"""Trainium2 Bass kernel for nn_InvariantHeadviaTP.

Reference computation (after dead-code elimination -- y1/y2/gates are never
used by the output):
    x0   = node_vec[:, :128]                  # [N, 128]
    a    = node_embedding                     # [N, 16]
    s0   = einsum('ni,na,iak->nk', x0, a, W1_l0[:, :, :128]) / sqrt(2048) + b1[:128]
    scal = silu(s0)                           # [N, 128]
    mid  = einsum('ni,na,iak->nk', scal, a, W2) / sqrt(2048) + b2   # [N, 16]
    h    = silu(mid @ W3 / 4 + b3)            # [N, 16]
    out  = h @ W4 / 4 + b4                    # [N, 1]

Strategy: data-parallel over 8 cores (2048 nodes each), transposed layout
(features on SBUF partitions, nodes on the free dim).

s0 contraction over c=(i,a) [128*16=2048] is chunked into 16 K=128 chunks
of (32 i's x 4 a's): chunk (q, c4) covers i in [32q, 32q+32), a in
[4*c4, 4*c4+4).  The moving operand for a chunk is
    u[p, n] = x0rep_q[p, n] * a4rep_c4[p, n]
      x0rep_q[p, n] = x0T[32q + p%32, n]   (x0 rows tiled 4x -> 2MB/core)
      a4rep_c4[p,n] = aT[4c4 + p//32, n]   (a rows repeated 32x -> 2MB/core)
built by one [128, 1024] DVE multiply per chunk per node-half (4x DVE mode).
This halves the broadcast DMA vs replicating aT across all 128 partitions
(8MB/core) while keeping all operands SBUF/bf16/contiguous.

mid path avoids 16 M=16 matmuls: P[(a,k), n] = sum_i W2[i,a,k] scal[i,n]
via 2 M=128 matmuls, gate-multiply by a16rep (a rows repeated 16x), then a
one-hot selector matmul sums the 8 a's per half: 8192 PE columns instead
of 32768.
"""

import numpy as np
import ml_dtypes
from contextlib import ExitStack

import concourse.bass as bass
import concourse.mybir as mybir
import concourse.tile as tile
from concourse import bacc
from concourse.bass import ts
from concourse.bass_utils import run_bass_kernel_spmd

N_CORES = 8
N_FULL = 16384
NSH = N_FULL // N_CORES          # 2048 nodes per core
A = 16                           # attr dim
M0 = 128                         # MUL0 (scalar channels)
HALF = 1024                      # node half per core
FT = 512                         # matmul moving free size (PSUM bank)
SCALE = 1.0 / np.sqrt(M0 * A)    # path normalization of both fctp einsums
BF16 = ml_dtypes.bfloat16

AF = mybir.ActivationFunctionType
F32 = mybir.dt.float32
DBF16 = mybir.dt.bfloat16


def build_nc(nsh: int = NSH, num_devices: int = N_CORES):
    assert nsh == 4 * FT, "kernel is laid out for 4 PSUM-bank node tiles"

    nc = bacc.Bacc(
        "TRN2",
        target_bir_lowering=False,
        debug=False,
        enable_asserts=False,
        num_devices=num_devices,
    )

    # node data (per-core shard, host-prepped layouts)
    x0rep = nc.dram_tensor("x0rep", [M0, 4 * nsh], DBF16, kind="ExternalInput").ap()
    a4rep = nc.dram_tensor("a4rep", [M0, 4 * nsh], DBF16, kind="ExternalInput").ap()
    a16rep = nc.dram_tensor("a16rep", [M0, 2 * nsh], DBF16, kind="ExternalInput").ap()
    # weights
    w0 = nc.dram_tensor("w0", [M0, 16 * M0], DBF16, kind="ExternalInput").ap()
    w2p = nc.dram_tensor("w2p", [M0, 2 * M0], DBF16, kind="ExternalInput").ap()
    selw3 = nc.dram_tensor("selw3", [M0, A], DBF16, kind="ExternalInput").ap()
    w4 = nc.dram_tensor("w4", [A, 1], DBF16, kind="ExternalInput").ap()
    b1 = nc.dram_tensor("b1", [M0, 1], F32, kind="ExternalInput").ap()
    b3p = nc.dram_tensor("b3p", [A, 1], F32, kind="ExternalInput").ap()
    b4 = nc.dram_tensor("b4", [1, 1], F32, kind="ExternalInput").ap()
    outt = nc.dram_tensor("outt", [1, nsh], F32, kind="ExternalOutput").ap()

    with tile.TileContext(nc) as tc, ExitStack() as ctx:
        consts = ctx.enter_context(tc.tile_pool(name="consts", bufs=1))

        # Node-data residents: one tile + one DMA per block so each chunk's
        # compute only waits for its own blocks. DMA issue costs ~650ns of
        # sequencer time per dma_start, so spread the issues across the
        # sync and scalar queues (both idle at kernel start; vector/gpsimd
        # cannot issue HWDGE DMAs / pay drain overhead).
        w0_sb = consts.tile([M0, 16 * M0], DBF16)
        w2p_sb = consts.tile([M0, 2 * M0], DBF16)
        selw3_sb = consts.tile([M0, A], DBF16)
        w4_sb = consts.tile([A, 1], DBF16)
        b1_sb = consts.tile([M0, 1], F32)
        b3p_sb = consts.tile([A, 1], F32)
        b4_sb = consts.tile([1, 1], F32)
        x0rep_sb = []
        a4rep_sb = []
        a16rep_sb = []
        for blk in range(4):
            t = consts.tile([M0, nsh], DBF16, name=f"x0rep_sb{blk}")
            x0rep_sb.append(t)
        for blk in range(4):
            t = consts.tile([M0, nsh], DBF16, name=f"a4rep_sb{blk}")
            a4rep_sb.append(t)
        for blk in range(2):
            t = consts.tile([M0, nsh], DBF16, name=f"a16rep_sb{blk}")
            a16rep_sb.append(t)
        # first-use order: chunk ci=(q,c4) uses x0[q], a4[c4]
        nc.sync.dma_start(w0_sb[:], w0)
        nc.scalar.dma_start(a4rep_sb[0][:], a4rep[:, ts(0, nsh)])
        nc.sync.dma_start(x0rep_sb[0][:], x0rep[:, ts(0, nsh)])
        nc.scalar.dma_start(a4rep_sb[1][:], a4rep[:, ts(1, nsh)])
        nc.sync.dma_start(a4rep_sb[2][:], a4rep[:, ts(2, nsh)])
        nc.scalar.dma_start(a4rep_sb[3][:], a4rep[:, ts(3, nsh)])
        nc.sync.dma_start(x0rep_sb[1][:], x0rep[:, ts(1, nsh)])
        nc.scalar.dma_start(x0rep_sb[2][:], x0rep[:, ts(2, nsh)])
        nc.sync.dma_start(x0rep_sb[3][:], x0rep[:, ts(3, nsh)])
        nc.scalar.dma_start(a16rep_sb[0][:], a16rep[:, ts(0, nsh)])
        nc.sync.dma_start(a16rep_sb[1][:], a16rep[:, ts(1, nsh)])
        nc.scalar.dma_start(w2p_sb[:], w2p)
        nc.sync.dma_start(selw3_sb[:], selw3)
        nc.scalar.dma_start(b1_sb[:], b1)
        nc.sync.dma_start(w4_sb[:], w4)
        nc.scalar.dma_start(b3p_sb[:], b3p)
        nc.sync.dma_start(b4_sb[:], b4)

        u_pool = ctx.enter_context(tc.tile_pool(name="u", bufs=3))
        s_pool = ctx.enter_context(tc.tile_pool(name="s", bufs=2))
        scal_pool = ctx.enter_context(tc.tile_pool(name="scal", bufs=4))
        pm_pool = ctx.enter_context(tc.tile_pool(name="pm", bufs=4))
        o_pool = ctx.enter_context(tc.tile_pool(name="o", bufs=2))
        # PSUM budget (8 banks): s0 4 tags + p 2 tags + mm 2 tags, bufs=1.
        ps_s0 = ctx.enter_context(tc.tile_pool(name="ps_s0", bufs=1, space="PSUM"))
        ps_p = ctx.enter_context(tc.tile_pool(name="ps_p", bufs=1, space="PSUM"))
        ps_mm = ctx.enter_context(tc.tile_pool(name="ps_mm", bufs=1, space="PSUM"))

        # ---- s0 accumulation: 16 chunks of K=128 = (32 i x 4 a), full
        # shard width per chunk; w0 lhsT loaded once per chunk. ----
        s0_ps = [
            ps_s0.tile([M0, FT], F32, tag=f"s0_{f}", name=f"s0_{f}")
            for f in range(4)
        ]
        for ci in range(16):
            q, c4 = ci // 4, ci % 4
            u = u_pool.tile([M0, nsh], DBF16, tag="u")
            nc.vector.tensor_mul(u[:], x0rep_sb[q][:], a4rep_sb[c4][:])
            for f in range(4):
                nc.tensor.matmul(
                    s0_ps[f][:],
                    w0_sb[:, ts(ci, M0)],
                    u[:, ts(f, FT)],
                    start=(ci == 0),
                    stop=(ci == 15),
                )

        # ---- epilogue in pairs of 512-node tiles; matmuls grouped by
        # lhsT so each weight is loaded once per pair.
        # mid is linear between the selector reduction and W3, so both fold
        # into one lhsT: selW3[p, j] = W3[p%16, j]/4, with bias
        # b3' = W3.T b2 / 4 + b3:  h_pre = selW3.T @ (Pm0 + Pm1) + b3'. ----
        for g in range(2):
            fs = [2 * g, 2 * g + 1]

            scal = {}
            for f in fs:
                s_sig = s_pool.tile([M0, FT], DBF16, tag="s_sig")
                nc.scalar.activation(s_sig[:], s0_ps[f][:], AF.Sigmoid, bias=b1_sb[:])
                s_idn = s_pool.tile([M0, FT], DBF16, tag="s_idn")
                nc.scalar.activation(s_idn[:], s0_ps[f][:], AF.Identity, bias=b1_sb[:])
                sc = scal_pool.tile([M0, FT], DBF16, tag=f"scal_{f % 2}",
                                    name=f"scal_{f}")
                nc.vector.tensor_mul(sc[:], s_idn[:], s_sig[:])
                scal[f] = sc

            # P[(a_l,k), n] for a-halves 0/1, gated by a16rep.
            mm = {}
            for f in fs:
                mm[f] = ps_mm.tile([65, FT], F32, tag=f"mm_{f % 2}", name=f"mm_{f}")
            pm_t = {}
            for ah in range(2):
                for f in fs:
                    p_ps = ps_p.tile([M0, FT], F32, tag=f"p_{f % 2}",
                                     name=f"p_{ah}_{f}")
                    nc.tensor.matmul(
                        p_ps[:], w2p_sb[:, ts(ah, M0)], scal[f][:],
                        start=True, stop=True,
                    )
                    pm = pm_pool.tile([M0, FT], DBF16, tag=f"pm_{ah}_{f % 2}",
                                      name=f"pm_{ah}_{f}")
                    nc.vector.tensor_mul(
                        pm[:], p_ps[:],
                        a16rep_sb[ah][:, ts(f, FT)],
                    )
                    pm_t[(ah, f)] = pm
            for f in fs:
                for ah in range(2):
                    nc.tensor.matmul(
                        mm[f][32:48, :], selw3_sb[:], pm_t[(ah, f)][:],
                        start=(ah == 0), stop=(ah == 1),
                    )

            hb = {}
            for f in fs:
                h_sig = s_pool.tile([A, FT], DBF16, tag="h_sig")
                nc.scalar.activation(h_sig[:], mm[f][32:48, :], AF.Sigmoid,
                                     bias=b3p_sb[:])
                h_idn = s_pool.tile([A, FT], DBF16, tag="h_idn")
                nc.scalar.activation(h_idn[:], mm[f][32:48, :], AF.Identity,
                                     bias=b3p_sb[:])
                hbt = s_pool.tile([A, FT], DBF16, tag="hb", name=f"hb_{f}")
                nc.vector.tensor_mul(hbt[:], h_idn[:], h_sig[:])
                hb[f] = hbt
            for f in fs:
                nc.tensor.matmul(mm[f][64:65, :], w4_sb[:], hb[f][:],
                                 start=True, stop=True)
            for f in fs:
                ob = o_pool.tile([1, FT], F32, tag="ob", name=f"ob_{f}")
                nc.scalar.activation(ob[:], mm[f][64:65, :], AF.Identity,
                                     bias=b4_sb[:])
                eng = nc.scalar if f % 2 == 0 else nc.sync
                eng.dma_start(outt[:, ts(f, FT)], ob[:])

    nc.compile()
    return nc


def prep_host(inputs: dict, nsh: int = NSH, n_cores: int = N_CORES):
    """Host-side prep: slice/transpose/cast inputs, build per-core in_maps."""
    node_vec = np.asarray(inputs["node_vec"], dtype=np.float32)
    node_embedding = np.asarray(inputs["node_embedding"], dtype=np.float32)
    W1_l0 = np.asarray(inputs["W1_l0"], dtype=np.float32)
    b1 = np.asarray(inputs["b1"], dtype=np.float32)
    W2 = np.asarray(inputs["W2"], dtype=np.float32)
    b2 = np.asarray(inputs["b2"], dtype=np.float32)
    W3 = np.asarray(inputs["W3"], dtype=np.float32)
    b3 = np.asarray(inputs["b3"], dtype=np.float32)
    W4 = np.asarray(inputs["W4"], dtype=np.float32)
    b4 = np.asarray(inputs["b4"], dtype=np.float32)

    x0T = np.ascontiguousarray(node_vec[:, :M0].T).astype(BF16)      # [128, N]
    aT = np.ascontiguousarray(node_embedding.T).astype(BF16)         # [16, N]

    # w0 chunk ci = (q, c4): [p, k] = W[32q + p%32, 4c4 + p//32, k]
    W = (W1_l0[:, :, :M0] * SCALE).astype(np.float32)                # [128,16,128]
    w0_blocks = []
    for ci in range(16):
        q, c4 = ci // 4, ci % 4
        blk = W[q * 32:(q + 1) * 32, c4 * 4:(c4 + 1) * 4, :]         # [32, 4, 128]
        w0_blocks.append(blk.transpose(1, 0, 2).reshape(M0, M0))     # p = a_l*32+i_l
    w0h = np.concatenate(w0_blocks, axis=1).astype(BF16)             # [128, 2048]

    w2ph = (W2 * SCALE).reshape(M0, A * A).astype(BF16)              # [128, 256]
    # selector+W3 fold: selW3[p, j] = W3[p%16, j]/4; b3' = W3.T b2/4 + b3
    selw3 = np.tile(W3 / np.sqrt(A), (8, 1)).astype(BF16)            # [128, 16]
    b3p = (W3.T @ b2) / np.sqrt(A) + b3                              # [16]
    w4h = (W4 / np.sqrt(A)).astype(BF16)

    shared = {
        "w0": w0h, "w2p": w2ph, "selw3": selw3, "w4": w4h,
        "b1": np.ascontiguousarray(b1[:M0].reshape(M0, 1)),
        "b3p": np.ascontiguousarray(b3p.reshape(A, 1)),
        "b4": np.ascontiguousarray(b4.reshape(1, 1)),
    }
    in_maps = []
    for c in range(n_cores):
        sl = slice(c * nsh, (c + 1) * nsh)
        x0s = x0T[:, sl]
        ats = aT[:, sl]
        x0rep = np.concatenate(
            [np.tile(x0s[q * 32:(q + 1) * 32, :], (4, 1)) for q in range(4)], axis=1
        )                                                            # [128, 4*nsh]
        a4rep = np.concatenate(
            [np.repeat(ats[c4 * 4:(c4 + 1) * 4, :], 32, axis=0) for c4 in range(4)],
            axis=1,
        )                                                            # [128, 4*nsh]
        a16rep = np.concatenate(
            [np.repeat(ats[ah * 8:(ah + 1) * 8, :], 16, axis=0) for ah in range(2)],
            axis=1,
        )                                                            # [128, 2*nsh]
        in_maps.append({
            "x0rep": np.ascontiguousarray(x0rep),
            "a4rep": np.ascontiguousarray(a4rep),
            "a16rep": np.ascontiguousarray(a16rep),
            **shared,
        })
    return in_maps


_NC_CACHE = {}


def _get_nc():
    if "nc" not in _NC_CACHE:
        _NC_CACHE["nc"] = build_nc()
    return _NC_CACHE["nc"]


def kernel_with_results(trace: bool = False, **inputs):
    nc = _get_nc()
    in_maps = prep_host(inputs)
    res = run_bass_kernel_spmd(
        nc, in_maps, core_ids=list(range(N_CORES)), trace=trace,
    )
    out = np.empty((N_FULL, 1), dtype=np.float32)
    for c in range(N_CORES):
        out[c * NSH:(c + 1) * NSH, 0] = res.results[c]["outt"][0]
    return out, res


def kernel(**inputs) -> np.ndarray:
    out, _ = kernel_with_results(trace=False, **inputs)
    return out


# revision 18
# speedup vs baseline: 2.1706x; 1.0991x over previous
"""Trainium2 Bass kernel for nn_InvariantHeadviaTP.

Reference computation (after dead-code elimination -- y1/y2/gates are never
used by the output):
    x0   = node_vec[:, :128]                  # [N, 128]
    a    = node_embedding                     # [N, 16]
    s0   = einsum('ni,na,iak->nk', x0, a, W1_l0[:, :, :128]) / sqrt(2048) + b1[:128]
    scal = silu(s0)                           # [N, 128]
    mid  = einsum('ni,na,iak->nk', scal, a, W2) / sqrt(2048) + b2   # [N, 16]
    h    = silu(mid @ W3 / 4 + b3)            # [N, 16]
    out  = h @ W4 / 4 + b4                    # [N, 1]

Strategy: data-parallel over 8 cores (2048 nodes each), transposed layout
(features on SBUF partitions, nodes on the free dim).

s0 contraction over c=(i,a) [128*16=2048] is chunked into 16 K=128 chunks
of (32 i's x 4 a's): chunk (q, c4) covers i in [32q, 32q+32), a in
[4c4, 4c4+4).  The moving operand for a chunk is
    u[p, n] = x0rep_q[p, n] * a4rep_c4[p, n]
      x0rep_q[p, n] = x0T[32q + p%32, n]   (x0 rows tiled 4x -> 2MB/core)
      a4rep_c4[p,n] = aT[4c4 + p//32, n]   (a rows repeated 32x -> 2MB/core)
This halves the broadcast DMA vs replicating aT across all 128 partitions
(8MB/core) while keeping all DVE operands SBUF/bf16/contiguous (2x mode).
u is built for q-PAIRS in one DVE op (a4rep read twice via a stride-0 AP).

mid path avoids 16 M=16 matmuls: P[(a,k), n] = sum_i W2[i,a,k] scal[i,n]
via 2 M=128 matmuls, gate-multiply by a16rep (a rows repeated 16x); mid is
linear between the one-hot selector reduction and W3, so both fold into
one lhsT selW3[p, j] = W3[p%16, j]/4 with bias b3' = W3.T b2/4 + b3.

silu is a single scalar-engine activation (HW act table `silu_and_others`;
CoreSim has no Silu LUT -- build with use_silu=False for simulation).
"""

import numpy as np
import ml_dtypes
from contextlib import ExitStack

import concourse.bass as bass
import concourse.mybir as mybir
import concourse.tile as tile
from concourse import bacc
from concourse.bass import ts
from concourse.bass_utils import run_bass_kernel_spmd

N_CORES = 8
N_FULL = 16384
NSH = N_FULL // N_CORES          # 2048 nodes per core
A = 16                           # attr dim
M0 = 128                         # MUL0 (scalar channels)
FT = 512                         # matmul moving free size (PSUM bank)
SCALE = 1.0 / np.sqrt(M0 * A)    # path normalization of both fctp einsums
BF16 = ml_dtypes.bfloat16

AF = mybir.ActivationFunctionType
F32 = mybir.dt.float32
DBF16 = mybir.dt.bfloat16

BFP_W2P = 0                      # bfpack column layout
BFP_SELW3 = 2 * M0
BFP_W4 = 2 * M0 + A
BFP_COLS = 2 * M0 + A + 1


def build_nc(nsh: int = NSH, num_devices: int = N_CORES, use_silu: bool = True):
    assert nsh == 4 * FT, "kernel is laid out for 4 PSUM-bank node tiles"

    nc = bacc.Bacc(
        "TRN2",
        target_bir_lowering=False,
        debug=False,
        enable_asserts=False,
        num_devices=num_devices,
    )

    # node data (per-core shard, host-prepped layouts)
    x0rep = nc.dram_tensor("x0rep", [M0, 4 * nsh], DBF16, kind="ExternalInput").ap()
    a4rep = nc.dram_tensor("a4rep", [M0, 4 * nsh], DBF16, kind="ExternalInput").ap()
    a16rep = nc.dram_tensor("a16rep", [M0, 2 * nsh], DBF16, kind="ExternalInput").ap()
    # weights: w0 + one bf16 pack + one f32 pack
    w0 = nc.dram_tensor("w0", [M0, 16 * M0], DBF16, kind="ExternalInput").ap()
    bfpack = nc.dram_tensor("bfpack", [M0, BFP_COLS], DBF16, kind="ExternalInput").ap()
    fpack = nc.dram_tensor("fpack", [M0, 3], F32, kind="ExternalInput").ap()
    outt = nc.dram_tensor("outt", [1, nsh], F32, kind="ExternalOutput").ap()

    with tile.TileContext(nc) as tc, ExitStack() as ctx:
        consts = ctx.enter_context(tc.tile_pool(name="consts", bufs=1))

        w0_sb = consts.tile([M0, 16 * M0], DBF16)
        bfpack_sb = consts.tile([M0, BFP_COLS], DBF16)
        fpack_sb = consts.tile([M0, 3], F32)
        x0big_sb = consts.tile([M0, 4 * nsh], DBF16)
        a4rep_sb = []
        a16rep_sb = []
        for blk in range(4):
            t = consts.tile([M0, nsh], DBF16, name=f"a4rep_sb{blk}")
            a4rep_sb.append(t)
        for blk in range(2):
            t = consts.tile([M0, nsh], DBF16, name=f"a16rep_sb{blk}")
            a16rep_sb.append(t)

        w2p_sb = bfpack_sb[:, BFP_SELW3 - 2 * M0:BFP_SELW3]  # [:, 0:256]
        selw3_sb = bfpack_sb[:, BFP_SELW3:BFP_W4]
        w4_sb = bfpack_sb[0:A, BFP_W4:BFP_W4 + 1]
        b1_sb = fpack_sb[:, 0:1]
        b3p_sb = fpack_sb[0:A, 1:2]
        b4_sb = fpack_sb[0:1, 2:3]

        # DMA issue costs ~650ns of sequencer time per dma_start, so the
        # issues are spread across the sync and scalar queues (idle at
        # start; scalar's first issue lands after its act-table load).
        # First-use order: chunks iterate c4-outer / q-pair-inner.
        nc.sync.dma_start(x0big_sb[:, ts(0, nsh)], x0rep[:, ts(0, nsh)])
        nc.scalar.dma_start(a4rep_sb[0][:], a4rep[:, ts(0, nsh)])
        nc.sync.dma_start(w0_sb[:], w0)
        nc.scalar.dma_start(x0big_sb[:, ts(1, nsh)], x0rep[:, ts(1, nsh)])
        nc.sync.dma_start(x0big_sb[:, ts(2, nsh)], x0rep[:, ts(2, nsh)])
        nc.scalar.dma_start(x0big_sb[:, ts(3, nsh)], x0rep[:, ts(3, nsh)])
        nc.sync.dma_start(a4rep_sb[1][:], a4rep[:, ts(1, nsh)])
        nc.scalar.dma_start(a4rep_sb[2][:], a4rep[:, ts(2, nsh)])
        nc.sync.dma_start(a4rep_sb[3][:], a4rep[:, ts(3, nsh)])
        nc.scalar.dma_start(a16rep_sb[0][:], a16rep[:, ts(0, nsh)])
        nc.sync.dma_start(a16rep_sb[1][:], a16rep[:, ts(1, nsh)])
        nc.scalar.dma_start(bfpack_sb[:], bfpack)
        nc.sync.dma_start(fpack_sb[:], fpack)

        u_pool = ctx.enter_context(tc.tile_pool(name="u", bufs=3))
        s_pool = ctx.enter_context(tc.tile_pool(name="s", bufs=2))
        scal_pool = ctx.enter_context(tc.tile_pool(name="scal", bufs=4))
        pm_pool = ctx.enter_context(tc.tile_pool(name="pm", bufs=4))
        o_pool = ctx.enter_context(tc.tile_pool(name="o", bufs=1))
        # PSUM budget (8 banks): s0 4 tags + p 2 tags + mm 2 tags, bufs=1.
        ps_s0 = ctx.enter_context(tc.tile_pool(name="ps_s0", bufs=1, space="PSUM"))
        ps_p = ctx.enter_context(tc.tile_pool(name="ps_p", bufs=1, space="PSUM"))
        ps_mm = ctx.enter_context(tc.tile_pool(name="ps_mm", bufs=1, space="PSUM"))

        ob_all = o_pool.tile([1, nsh], F32)

        # ---- s0 accumulation: 16 chunks of K=128 = (32 i x 4 a); u built
        # per q-pair with one DVE op (a4rep block repeated via 0-stride AP),
        # w0 lhsT loaded once per chunk. ----
        s0_ps = [
            ps_s0.tile([M0, FT], F32, tag=f"s0_{f}", name=f"s0_{f}")
            for f in range(4)
        ]
        for c4 in range(4):
            a4b = a4rep_sb[c4][:].unsqueeze(1).broadcast_to([M0, 2, nsh])
            for qp in range(2):
                u = u_pool.tile([M0, 2 * nsh], DBF16, tag="u")
                nc.vector.tensor_mul(
                    u[:], x0big_sb[:, bass.ds(qp * 2 * nsh, 2 * nsh)], a4b
                )
                for qloc in range(2):
                    q = qp * 2 + qloc
                    ci = q * 4 + c4  # w0 host block index
                    for f in range(4):
                        nc.tensor.matmul(
                            s0_ps[f][:],
                            w0_sb[:, ts(ci, M0)],
                            u[:, bass.ds(qloc * nsh + f * FT, FT)],
                            start=(c4 == 0 and q == 0),
                            stop=(c4 == 3 and q == 3),
                        )

        # ---- epilogue in pairs of 512-node tiles; matmuls grouped by
        # lhsT so each weight is loaded once per pair. ----
        for g in range(2):
            fs = [2 * g, 2 * g + 1]

            scal = {}
            for f in fs:
                sc = scal_pool.tile([M0, FT], DBF16, tag=f"scal_{f % 2}",
                                    name=f"scal_{f}")
                if use_silu:
                    nc.scalar.activation(sc[:], s0_ps[f][:], AF.Silu, bias=b1_sb)
                else:
                    s_sig = s_pool.tile([M0, FT], DBF16, tag="s_sig")
                    nc.scalar.activation(s_sig[:], s0_ps[f][:], AF.Sigmoid,
                                         bias=b1_sb)
                    s_idn = s_pool.tile([M0, FT], DBF16, tag="s_idn")
                    nc.scalar.activation(s_idn[:], s0_ps[f][:], AF.Identity,
                                         bias=b1_sb)
                    nc.vector.tensor_mul(sc[:], s_idn[:], s_sig[:])
                scal[f] = sc

            # P[(a_l,k), n] for a-halves 0/1, gated by a16rep.
            mm = {}
            for f in fs:
                mm[f] = ps_mm.tile([65, FT], F32, tag=f"mm_{f % 2}", name=f"mm_{f}")
            pm_t = {}
            for ah in range(2):
                for f in fs:
                    p_ps = ps_p.tile([M0, FT], F32, tag=f"p_{f % 2}",
                                     name=f"p_{ah}_{f}")
                    nc.tensor.matmul(
                        p_ps[:], w2p_sb[:, ts(ah, M0)], scal[f][:],
                        start=True, stop=True,
                    )
                    pm = pm_pool.tile([M0, FT], DBF16, tag=f"pm_{ah}_{f % 2}",
                                      name=f"pm_{ah}_{f}")
                    nc.vector.tensor_mul(
                        pm[:], p_ps[:],
                        a16rep_sb[ah][:, ts(f, FT)],
                    )
                    pm_t[(ah, f)] = pm
            for f in fs:
                for ah in range(2):
                    nc.tensor.matmul(
                        mm[f][32:48, :], selw3_sb, pm_t[(ah, f)][:],
                        start=(ah == 0), stop=(ah == 1),
                    )

            hb = {}
            for f in fs:
                hbt = s_pool.tile([A, FT], DBF16, tag="hb", name=f"hb_{f}")
                if use_silu:
                    nc.scalar.activation(hbt[:], mm[f][32:48, :], AF.Silu,
                                         bias=b3p_sb)
                else:
                    h_sig = s_pool.tile([A, FT], DBF16, tag="h_sig")
                    nc.scalar.activation(h_sig[:], mm[f][32:48, :], AF.Sigmoid,
                                         bias=b3p_sb)
                    h_idn = s_pool.tile([A, FT], DBF16, tag="h_idn")
                    nc.scalar.activation(h_idn[:], mm[f][32:48, :], AF.Identity,
                                         bias=b3p_sb)
                    nc.vector.tensor_mul(hbt[:], h_idn[:], h_sig[:])
                hb[f] = hbt
            for f in fs:
                nc.tensor.matmul(mm[f][64:65, :], w4_sb, hb[f][:],
                                 start=True, stop=True)
            for f in fs:
                nc.scalar.activation(ob_all[0:1, ts(f, FT)], mm[f][64:65, :],
                                     AF.Identity, bias=b4_sb)

        nc.sync.dma_start(outt, ob_all[:])

    nc.compile()
    return nc


def prep_host(inputs: dict, nsh: int = NSH, n_cores: int = N_CORES):
    """Host-side prep: slice/transpose/cast inputs, build per-core in_maps."""
    node_vec = np.asarray(inputs["node_vec"], dtype=np.float32)
    node_embedding = np.asarray(inputs["node_embedding"], dtype=np.float32)
    W1_l0 = np.asarray(inputs["W1_l0"], dtype=np.float32)
    b1 = np.asarray(inputs["b1"], dtype=np.float32)
    W2 = np.asarray(inputs["W2"], dtype=np.float32)
    b2 = np.asarray(inputs["b2"], dtype=np.float32)
    W3 = np.asarray(inputs["W3"], dtype=np.float32)
    b3 = np.asarray(inputs["b3"], dtype=np.float32)
    W4 = np.asarray(inputs["W4"], dtype=np.float32)
    b4 = np.asarray(inputs["b4"], dtype=np.float32)

    x0T = np.ascontiguousarray(node_vec[:, :M0].T).astype(BF16)      # [128, N]
    aT = np.ascontiguousarray(node_embedding.T).astype(BF16)         # [16, N]

    # w0 chunk ci = (q, c4): [p, k] = W[32q + p%32, 4c4 + p//32, k]
    W = (W1_l0[:, :, :M0] * SCALE).astype(np.float32)                # [128,16,128]
    w0_blocks = []
    for ci in range(16):
        q, c4 = ci // 4, ci % 4
        blk = W[q * 32:(q + 1) * 32, c4 * 4:(c4 + 1) * 4, :]         # [32, 4, 128]
        w0_blocks.append(blk.transpose(1, 0, 2).reshape(M0, M0))     # p = a_l*32+i_l
    w0h = np.concatenate(w0_blocks, axis=1).astype(BF16)             # [128, 2048]

    w2ph = (W2 * SCALE).reshape(M0, A * A)                           # [128, 256]
    # selector+W3 fold: selW3[p, j] = W3[p%16, j]/4; b3' = W3.T b2/4 + b3
    selw3 = np.tile(W3 / np.sqrt(A), (8, 1))                         # [128, 16]
    b3p = (W3.T @ b2) / np.sqrt(A) + b3                              # [16]
    w4h = W4 / np.sqrt(A)                                            # [16, 1]

    bfpack = np.zeros((M0, BFP_COLS), dtype=np.float32)
    bfpack[:, 0:2 * M0] = w2ph
    bfpack[:, BFP_SELW3:BFP_W4] = selw3
    bfpack[0:A, BFP_W4] = w4h[:, 0]
    bfpack = bfpack.astype(BF16)

    fpack = np.zeros((M0, 3), dtype=np.float32)
    fpack[:, 0] = b1[:M0]
    fpack[0:A, 1] = b3p
    fpack[0, 2] = b4[0]

    shared = {"w0": w0h, "bfpack": bfpack, "fpack": fpack}
    in_maps = []
    for c in range(n_cores):
        sl = slice(c * nsh, (c + 1) * nsh)
        x0s = x0T[:, sl]
        ats = aT[:, sl]
        x0rep = np.concatenate(
            [np.tile(x0s[q * 32:(q + 1) * 32, :], (4, 1)) for q in range(4)], axis=1
        )                                                            # [128, 4*nsh]
        a4rep = np.concatenate(
            [np.repeat(ats[c4 * 4:(c4 + 1) * 4, :], 32, axis=0) for c4 in range(4)],
            axis=1,
        )                                                            # [128, 4*nsh]
        a16rep = np.concatenate(
            [np.repeat(ats[ah * 8:(ah + 1) * 8, :], 16, axis=0) for ah in range(2)],
            axis=1,
        )                                                            # [128, 2*nsh]
        in_maps.append({
            "x0rep": np.ascontiguousarray(x0rep),
            "a4rep": np.ascontiguousarray(a4rep),
            "a16rep": np.ascontiguousarray(a16rep),
            **shared,
        })
    return in_maps


_NC_CACHE = {}


def _get_nc():
    if "nc" not in _NC_CACHE:
        _NC_CACHE["nc"] = build_nc()
    return _NC_CACHE["nc"]


def kernel_with_results(trace: bool = False, **inputs):
    nc = _get_nc()
    in_maps = prep_host(inputs)
    res = run_bass_kernel_spmd(
        nc, in_maps, core_ids=list(range(N_CORES)), trace=trace,
    )
    out = np.empty((N_FULL, 1), dtype=np.float32)
    for c in range(N_CORES):
        out[c * NSH:(c + 1) * NSH, 0] = res.results[c]["outt"][0]
    return out, res


def kernel(**inputs) -> np.ndarray:
    out, _ = kernel_with_results(trace=False, **inputs)
    return out
